# revision 14
# baseline (speedup 1.0000x reference)
"""Trainium2 Bass kernel for nn_ChannelSelfAttention.

Reference computation (per batch sample b):
    xt   = x[b].T                          # [C, L]
    q    = xt @ Wq.T + bq                  # [C, H]
    kv   = xt @ Wkv.T + bkv                # [C, 2H] -> k, v
    attn = (q * H**-0.5) @ k.T             # [C, C]  (no softmax)
    y    = attn @ v                        # [C, H]
    g    = mean(y, axis=-1)                # [C]
    out[b] = x[b] * g[None, :]             # [L, C]

Sharding: data-parallel over B across 8 cores (4 samples per core);
weights replicated.

HBM-bound problem with a 2e-2 rel-err gate, so all HBM I/O is bf16
(host casts inputs, kernel stores bf16, host upcasts the output):
17.5 MiB per core vs 35 MiB in f32 -> ~46 us at the ~420 GB/s
single-ring DMA rate.

Device-side structure:
  - Tile-framework dependencies are TILE-granular, so x / W^T / out are
    split into per-l-chunk tiles (512 KiB / 384 KiB / 512 KiB DMAs,
    4 KiB descriptors): each qkv chunk-matmul gates only on its own
    chunk's DMA, and each output chunk stores as soon as its gate
    multiply finishes.
  - All loads ride the sync HWDGE ring, interleaved in consumption
    order (wT0, x0c0, wT1, x0c1, ... then x1..x3); constants ride the
    scalar ring (a few KiB).  Stores also ride the sync ring, emitted
    after all loads: ring FIFO then costs nothing (total bytes / rate
    is unchanged) and, critically, store issues do NOT occupy the ACT
    engine, whose in-order stream would otherwise stall the PSUM->SBUF
    copies of later samples behind store-gate semaphore waits.
  - qkv is computed x-stationary: lhsT = x chunk [128 l, 128 c-group],
    rhs = W_all^T chunk [128 l, 192]; 2 groups x 32 chunks x 192
    streamed columns = 12288 PE cycles/sample (the MAC optimum), and
    q, k, v land in natural [c, h] layout.  The bias (Wq and bq
    pre-scaled by H^-0.5 on the host) enters as a K=1 outer-product
    matmul that opens each PSUM accumulation group.
  - mean-over-H commutes into v: g[c] = sum_d attn^T[d,c] * vbar[d],
    vbar = mean_h v, so y is never materialized.  vbar comes from a
    DVE free-axis reduce of natural v; 1/H is folded into the ones
    lhsT of the final broadcast matmul, which lands g on all 128
    partitions.
  - q^T/k^T via 4 PE transposes; attn^T = k^T-chunk x q^T; at_sb =
    attn^T * vbar via DVE tensor_scalar (PSUM -> bf16 SBUF).
  - gate: per-chunk DVE tensor_tensor, all-bf16 (packed 2x mode), with
    g broadcast along j via a stride-0 AP.
  - PE warm-up: ~32 junk matmuls on a memset scratch right after the
    preamble keep the HAM activity monitor busy so the PE clock is at
    2.4 GHz (not the cold 1.2 GHz) when the first real matmul issues.
"""

import numpy as np
import ml_dtypes

import concourse.bass as bass
import concourse.mybir as mybir
import concourse.tile as tile
from concourse import bacc
from concourse.bass_utils import run_bass_kernel_spmd

B, L, C, H = 32, 4096, 256, 64
N_CORES = 8
B_LOC = B // N_CORES          # samples per core
P = 128                       # SBUF partitions
JC = 8                        # L-rows per partition per chunk (4KB bf16 descs)
NCH = L // (P * JC)           # l-chunks per sample (4)
GC = C // P                   # c-groups (2)
TH = 3 * H                    # 192 = q|k|v
BF16 = mybir.dt.bfloat16
F32 = mybir.dt.float32
SCALE = float(H) ** -0.5
BF = ml_dtypes.bfloat16
N_WARM = 18                   # PE warm-up junk matmuls


def _emit(tc: "tile.TileContext", x_d, wT_d, bias_d, id_d, ones1_d,
          onesg_d, out_d) -> None:
    nc = tc.nc
    with (
        tc.tile_pool(name="singles", bufs=1) as singles,
        tc.tile_pool(name="xin", bufs=B_LOC) as xin,
        tc.tile_pool(name="xout", bufs=3) as xout,
        tc.tile_pool(name="small", bufs=2) as small,
        tc.tile_pool(name="psA", bufs=2, space="PSUM") as psA,
        tc.tile_pool(name="psA2", bufs=2, space="PSUM") as psA2,
        tc.tile_pool(name="psB", bufs=2, space="PSUM") as psB,
        tc.tile_pool(name="psC", bufs=2, space="PSUM") as psC,
    ):
        # ---- constants on the scalar ring (a few KiB, land early) ----
        bias_sb = singles.tile([1, TH], BF16)                # (bq*scale)|bkv
        nc.scalar.dma_start(out=bias_sb, in_=bias_d[:])
        ones1 = singles.tile([1, P], BF16)                   # ones row
        nc.scalar.dma_start(out=ones1, in_=ones1_d[:])
        ident = singles.tile([P, P], BF16)
        nc.scalar.dma_start(out=ident, in_=id_d[:])
        onesg = singles.tile([P, P], BF16)                   # filled with 1/H
        nc.scalar.dma_start(out=onesg, in_=onesg_d[:])

        # ---- PE warm-up: junk matmuls on zeroed scratch so the HAM
        # clock-gate is at 2.4 GHz when real work arrives ----
        scratch = singles.tile([P, P + C], BF16)
        nc.vector.memset(scratch, 0.0)
        psj = psA.tile([P, TH], F32, tag="qkv0", name="psj")
        for _ in range(N_WARM):
            nc.tensor.matmul(psj, lhsT=scratch[:, 0:P],
                             rhs=scratch[:, P : P + TH])

        # ---- loads on the sync ring, in consumption order ----
        wT_src = wT_d[:].rearrange("(n p j) h -> n p (j h)", p=P, j=JC)
        x_srcs = [x_d[b].rearrange("(n p j) c -> n p (j c)", p=P, j=JC)
                  for b in range(B_LOC)]
        out_dsts = [out_d[b].rearrange("(n p j) c -> n p (j c)", p=P, j=JC)
                    for b in range(B_LOC)]
        wt0 = singles.tile([P, 1, JC * TH], BF16)            # 384 KiB
        nc.scalar.dma_start(out=wt0, in_=wT_src[0:1].rearrange("n p x -> p n x"))
        wtR = singles.tile([P, NCH - 1, JC * TH], BF16)      # 1.1 MiB
        nc.scalar.dma_start(
            out=wtR, in_=wT_src[1:NCH].rearrange("n p x -> p n x")
        )
        wts = [wt0[:, 0]] + [wtR[:, n - 1] for n in range(1, NCH)]
        xs = [[None] * 2 for _ in range(B_LOC)]
        for b in range(B_LOC):
            for h in range(2):
                xs[b][h] = xin.tile([P, NCH // 2, JC * C], BF16,
                                    tag=f"x{h}", name=f"x_b{b}_h{h}")
                eng = nc.scalar if h == 0 else nc.sync
                eng.dma_start(
                    out=xs[b][h],
                    in_=x_srcs[b][h * (NCH // 2) : (h + 1) * (NCH // 2)]
                    .rearrange("n p x -> p n x"),
                )

        def qkv_stage(b):
            pq = [psA.tile([P, TH], F32, tag="qkv0", name="pq0"),
                  psA2.tile([P, TH], F32, tag="qkv1", name="pq1")]
            for g in range(GC):
                nc.tensor.matmul(
                    pq[g], lhsT=ones1, rhs=bias_sb, start=True, stop=False,
                )
            for n in range(NCH):
                for j in range(JC):
                    for g in range(GC):
                        nc.tensor.matmul(
                            pq[g],
                            lhsT=xs[b][n // 2][
                                :, n % 2,
                                j * C + g * P : j * C + (g + 1) * P],
                            rhs=wts[n][:, j * TH : (j + 1) * TH],
                            start=False,
                            stop=(n == NCH - 1 and j == JC - 1),
                        )
            qkv_sb = small.tile([P, GC, TH], BF16, tag="qkv_sb")
            for g in range(GC):
                nc.scalar.copy(qkv_sb[:, g], pq[g])
            return qkv_sb

        def tail_stage(b, qkv_sb):
            # vbar[d] = sum_h v[d, h]  (1/H folded into onesg)
            vbar_sb = small.tile([P, GC, 1], F32, tag="vbar")
            for g in range(GC):
                nc.vector.tensor_reduce(
                    out=vbar_sb[:, g], in_=qkv_sb[:, g, 2 * H : TH],
                    axis=mybir.AxisListType.X, op=mybir.AluOpType.add,
                )
            # q^T, k^T [64, 256] via PE transpose
            psum_t = psB.tile([H, 2, C], BF16, tag="qkt")
            for g in range(GC):
                nc.tensor.transpose(
                    psum_t[:, 0, g * P : (g + 1) * P],
                    qkv_sb[:, g, 0:H], ident,
                )
                nc.tensor.transpose(
                    psum_t[:, 1, g * P : (g + 1) * P],
                    qkv_sb[:, g, H : 2 * H], ident,
                )
            qkt_sb = small.tile([H, 2, C], BF16, tag="qkt_sb")
            nc.scalar.copy(qkt_sb, psum_t)
            qT = qkt_sb[:, 0]
            kT = qkt_sb[:, 1]

            # attn^T[d, c] = sum_h k^T[h, d] q^T[h, c]
            psum_at = psC.tile([P, GC, C], F32, tag="at")
            for d in range(GC):
                nc.tensor.matmul(
                    psum_at[:, d], lhsT=kT[:, d * P : (d + 1) * P], rhs=qT,
                )
            # at_sb = attn^T * vbar (per-partition scalar), PSUM -> bf16
            at_sb = small.tile([P, GC, C], BF16, tag="at_sb")
            for d in range(GC):
                nc.scalar.activation(
                    out=at_sb[:, d], in_=psum_at[:, d],
                    func=mybir.ActivationFunctionType.Copy,
                    scale=vbar_sb[:, d],
                )
            # g[c] = (1/H) sum_d at_sb[d, c], broadcast to 128 partitions.
            # Reuses the at PSUM region (its values were already drained to
            # at_sb by the activations; Tile orders the write-after-read).
            psum_g = psum_at[:, 0]
            for d in range(GC):
                nc.tensor.matmul(
                    psum_g, lhsT=onesg, rhs=at_sb[:, d],
                    start=(d == 0), stop=(d == GC - 1),
                )
            g_sb = small.tile([P, C], BF16, tag="g_sb")
            nc.scalar.copy(g_sb, psum_g)

            # gate + store per chunk (each store flows as soon as its
            # chunk's gate multiply is done)
            g_bc = bass.AP(
                tensor=g_sb.tensor,
                offset=g_sb.offset,
                ap=[list(g_sb.ap[0]), [0, NCH // 2], [0, JC], list(g_sb.ap[1])],
            )
            for h in range(2):
                o_t = xout.tile([P, NCH // 2, JC * C], BF16,
                                tag=f"o{h}", name=f"o_b{b}_h{h}")
                nc.vector.tensor_tensor(
                    out=o_t.rearrange("p n (j c) -> p n j c", j=JC),
                    in0=xs[b][h].rearrange("p n (j c) -> p n j c", j=JC),
                    in1=g_bc,
                    op=mybir.AluOpType.mult,
                )
                nc.sync.dma_start(
                    out=out_dsts[b][h * (NCH // 2) : (h + 1) * (NCH // 2)]
                    .rearrange("n p x -> p n x"),
                    in_=o_t,
                )

        for b in range(B_LOC):
            tail_stage(b, qkv_stage(b))


def build():
    nc = bacc.Bacc(
        "TRN2", target_bir_lowering=False, debug=False, num_devices=N_CORES
    )
    x_d = nc.dram_tensor("x", [B_LOC, L, C], BF16, kind="ExternalInput")
    wT_d = nc.dram_tensor("wT", [L, TH], BF16, kind="ExternalInput")
    bias_d = nc.dram_tensor("bias", [1, TH], BF16, kind="ExternalInput")
    id_d = nc.dram_tensor("ident", [P, P], BF16, kind="ExternalInput")
    ones1_d = nc.dram_tensor("ones1", [1, P], BF16, kind="ExternalInput")
    onesg_d = nc.dram_tensor("onesg", [P, P], BF16, kind="ExternalInput")
    out_d = nc.dram_tensor("out", [B_LOC, L, C], BF16, kind="ExternalOutput")
    with tile.TileContext(nc) as tc:
        _emit(tc, x_d, wT_d, bias_d, id_d, ones1_d, onesg_d, out_d)
    nc.compile()
    return nc


_nc_cache = None


def _get_nc():
    global _nc_cache
    if _nc_cache is None:
        _nc_cache = build()
    return _nc_cache


def make_in_maps(x, Wq, bq, Wkv, bkv):
    x_bf = np.asarray(x, dtype=np.float32).astype(BF)
    wT = np.ascontiguousarray(
        np.concatenate(
            [np.asarray(Wq, np.float32) * SCALE, np.asarray(Wkv, np.float32)],
            axis=0,
        ).T.astype(BF)
    )
    bias = np.concatenate(
        [np.asarray(bq, np.float32) * SCALE, np.asarray(bkv, np.float32)]
    )[None].astype(BF)
    ident = np.eye(P, dtype=BF)
    ones1 = np.ones((1, P), dtype=BF)
    onesg = np.full((P, P), 1.0 / H, dtype=BF)
    return [
        {
            "x": np.ascontiguousarray(x_bf[i * B_LOC : (i + 1) * B_LOC]),
            "wT": wT,
            "bias": bias,
            "ident": ident,
            "ones1": ones1,
            "onesg": onesg,
        }
        for i in range(N_CORES)
    ]


def run(inputs, **spmd_kwargs):
    """Run on hardware; returns (full_output, BassKernelResults)."""
    nc = _get_nc()
    in_maps = make_in_maps(**inputs)
    res = run_bass_kernel_spmd(nc, in_maps, list(range(N_CORES)), **spmd_kwargs)
    out = np.concatenate([r["out"] for r in res.results], axis=0)
    return np.asarray(out).astype(np.float32), res


def kernel(**inputs) -> np.ndarray:
    out, _ = run(inputs)
    return out
